# revision 13
# baseline (speedup 1.0000x reference)
"""Trainium2 Bass kernel for nn_BeliefUpdate.

Reference semantics (LR=1.0): 10 iterations of
    belief <- softmax(log(belief + eps) + log_lik),
gated by a global convergence flag (done once max|change| < 1e-4).

Key identities used here:
  * With LR=1.0 the update is exactly `belief <- softmax(...)`, and the
    ungated 10-step recursion collapses to a closed form:
        e = (prior + eps) * exp(10 * (ln(lik+eps) + ln(m0+eps) + ln(m1+eps)))
        belief = e / rowsum(e)
    (no row-max subtraction needed: the per-row max exponent is far above
    the fp32 underflow cliff, so only absolutely-negligible entries flush
    to zero).  The eps re-additions of iterations 2..10 perturb the result
    at the ~1e-10 absolute scale for this input distribution (verified
    against the reference loop: absmax ~4e-7).
  * The done-gate only freezes belief AFTER the first iteration whose
    global max|change| < 1e-4.  A max over any row subset lower-bounds the
    global max, so a cheap host-side replay of 1024 rows proves "never
    converges" (subset changes are ~0.1 at every iteration, three orders
    of magnitude above the threshold).  If that check were ever
    inconclusive, a full host replay computes the exact gated result.

Sharding: pure data parallel over the batch dim across 8 cores; no
cross-core communication is needed (the allreduce-max is subsumed by the
host-side subset argument above).

Environment quirk: this walrus build caps each instruction at TWO sync
commands (waits + updates) per instruction.  Tile freely attaches more,
so `legalize_sync_budget` post-processes the scheduled program: excess
waits are hoisted onto freshly inserted same-engine NoOps immediately
before the instruction (same engine => waits still execute, in order,
before the instruction issues — semantics unchanged).
"""

import numpy as np

import concourse.bass as bass
import concourse.tile as tile
from concourse import mybir
from concourse.bass_utils import run_bass_kernel_spmd

F32 = mybir.dt.float32
AF = mybir.ActivationFunctionType
ALU = mybir.AluOpType

B, D = 131072, 256
NCORES = 8
SHARD = B // NCORES  # 16384 rows per core
EPS = 1e-10
NUM_ITERS = 10
THRESH = 1e-4
LR = 1.0

CHUNK = 1024  # rows per processing chunk
P = 128  # SBUF partitions

MAX_WAITS = 1  # this walrus build: at most 1 sem wait per instruction
MAX_UPDATES = 1  # ... and at most 1 sem update per instruction


def legalize_sync_budget(nc):
    """Hoist excess sem waits onto same-engine NoOps inserted before the
    offending instruction, so no instruction carries more than MAX_WAITS
    waits.  Engine program order makes this semantics-preserving: the
    NoOp's wait completes before the instruction issues."""
    counter = 0
    for f in nc.m.functions:
        for bb in f.blocks:
            insts = bb.instructions
            out = []
            changed = False
            for ins in insts:
                si = ins.sync_info
                if si is not None:
                    waits = list(si.on_wait or [])
                    ups = list(si.on_update or [])
                    if len(ups) > MAX_UPDATES:
                        raise RuntimeError(
                            f"{ins.name}: {len(ups)} sem updates cannot be legalized"
                        )
                    if len(waits) > MAX_WAITS:
                        hoist = waits[: len(waits) - MAX_WAITS]
                        kept = waits[len(waits) - MAX_WAITS :]
                        for w in hoist:
                            nop = mybir.InstNoOp(
                                name=f"sync-legalize-{counter}",
                                sync_info=mybir.SyncInfo(on_wait=[w], on_update=[]),
                                bass_nofuse=True,
                                engine=ins.engine,
                            )
                            counter += 1
                            out.append(nop)
                        ins.sync_info = mybir.SyncInfo(on_wait=kept, on_update=ups)
                        changed = True
                out.append(ins)
            if changed:
                bb.instructions = out
    return counter


def build_nc(shard_rows=SHARD, chunk_rows=CHUNK, legalize=True, repeats=1):
    """Build the Bass program for one core's shard.

    legalize=False skips the sync-budget post-pass (CoreSim requires every
    instruction to carry engine-sem updates, which the inserted NoOps
    don't; simulation correctness is unaffected by the pass anyway).

    repeats>1 emits the whole (idempotent) computation that many times in
    one NEFF — used to measure steady-state kernel time as a wall-clock
    delta between repeat counts, cancelling dispatch/transfer overheads.
    """
    assert shard_rows % chunk_rows == 0 and chunk_rows % P == 0
    tpc = chunk_rows // P  # rows per partition per chunk
    fd = tpc * D  # free-dim elements per big tile
    nchunks = shard_rows // chunk_rows

    nc = bass.Bass("TRN2", target_bir_lowering=False, debug=False)
    prior = nc.dram_tensor("prior", [shard_rows, D], F32, kind="ExternalInput").ap()
    lik = nc.dram_tensor("likelihood", [shard_rows, D], F32, kind="ExternalInput").ap()
    msgs = nc.dram_tensor("messages", [2, shard_rows, D], F32, kind="ExternalInput").ap()
    belief = nc.dram_tensor("belief", [shard_rows, D], F32, kind="ExternalOutput").ap()

    def dview(ap2d, c):
        # rows [c*chunk, (c+1)*chunk) -> [P, tpc, D]; partition p holds tpc
        # consecutive rows (contiguous tpc KB per partition: max DMA BW, and
        # exactly one DMACopy instruction per logical transfer)
        return ap2d[c * chunk_rows : (c + 1) * chunk_rows, :].rearrange(
            "(p t) d -> p t d", p=P
        )

    with tile.TileContext(nc) as tc:
        with (
            tc.tile_pool(name="const", bufs=1) as const,
            tc.tile_pool(name="io", bufs=2) as io,
            tc.tile_pool(name="work", bufs=2) as work,
            tc.tile_pool(name="scal", bufs=2) as scal,
        ):
            eps_t = const.tile([P, 1], F32)
            nc.vector.memset(eps_t[:], EPS)
            # Warm up the ACT tables (Ln/Exp) on low-sync dummies so walrus'
            # PSEUDO_LOAD_ACT_FUNC_SET lands on instructions with spare sync
            # slots.
            dummy = const.tile([P, 1], F32, tag="dummy")
            nc.scalar.activation(dummy[:], eps_t[:], AF.Ln, bias=eps_t[:])
            nc.scalar.activation(dummy[:], eps_t[:], AF.Exp, scale=float(NUM_ITERS))

            for c_rep in range(repeats * nchunks):
                c = c_rep % nchunks
                t_lik = io.tile([P, fd], F32, tag="lik")
                t_m0 = io.tile([P, fd], F32, tag="m0")
                t_m1 = io.tile([P, fd], F32, tag="m1")
                t_pr = io.tile([P, fd], F32, tag="pr")
                v = lambda t: t.rearrange("p (t d) -> p t d", d=D)
                nc.sync.dma_start(out=v(t_lik), in_=dview(lik, c))
                nc.sync.dma_start(out=v(t_m0), in_=dview(msgs[0], c))
                nc.sync.dma_start(out=v(t_m1), in_=dview(msgs[1], c))
                nc.sync.dma_start(out=v(t_pr), in_=dview(prior, c))

                la = work.tile([P, fd], F32, tag="A")
                lb = work.tile([P, fd], F32, tag="B")
                lc = work.tile([P, fd], F32, tag="C")
                lg = work.tile([P, fd], F32, tag="G")
                le = work.tile([P, fd], F32, tag="E")
                lo = work.tile([P, fd], F32, tag="O")
                # logs (ScalarE, fused +eps via bias)
                nc.scalar.activation(la[:], t_lik[:], AF.Ln, bias=eps_t[:])
                nc.scalar.activation(lb[:], t_m0[:], AF.Ln, bias=eps_t[:])
                nc.scalar.activation(lc[:], t_m1[:], AF.Ln, bias=eps_t[:])
                # lsum = la + (lb + lc): one add on GPSIMD, one on DVE
                nc.gpsimd.tensor_add(lg[:], lb[:], lc[:])
                nc.vector.tensor_add(la[:], la[:], lg[:])
                # E = exp(10 * lsum)  (ScalarE, fused scale)
                nc.scalar.activation(le[:], la[:], AF.Exp, scale=float(NUM_ITERS))
                # e = (prior + eps) * E, in place over le, with fused per-row
                # sums (one row per partition per 256-wide slice)
                svec = scal.tile([P, tpc], F32, tag="s")
                rvec = scal.tile([P, tpc], F32, tag="r")
                for t in range(tpc):
                    sl = slice(t * D, (t + 1) * D)
                    nc.vector.scalar_tensor_tensor(
                        le[:, sl],
                        t_pr[:, sl],
                        EPS,
                        le[:, sl],
                        op0=ALU.add,
                        op1=ALU.mult,
                        accum_out=svec[:, t : t + 1],
                    )
                nc.vector.reciprocal(rvec[:], svec[:])
                for t in range(tpc):
                    sl = slice(t * D, (t + 1) * D)
                    nc.vector.tensor_scalar_mul(lo[:, sl], le[:, sl], rvec[:, t : t + 1])
                nc.sync.dma_start(out=dview(belief, c), in_=v(lo))
    if legalize:
        legalize_sync_budget(nc)
        bad = validate_sync_budget(nc)
        if bad:
            raise RuntimeError(f"sync budget still exceeded after legalize: {bad[:10]}")
    return nc


def validate_sync_budget(nc):
    """Walk the BIR JSON and return instructions exceeding the per-
    instruction sync budget (MAX_WAITS waits, MAX_UPDATES updates)."""
    bad = []
    j = nc.to_json()

    def walk(node):
        if isinstance(node, dict):
            if node.get("opcode") and "sync_info" in node:
                si = node["sync_info"] or {}
                nw = len(si.get("on_wait") or [])
                nu = len(si.get("on_update") or [])
                if nw > MAX_WAITS or nu > MAX_UPDATES:
                    bad.append((node.get("name"), node.get("opcode"), nw, nu))
            for v in node.values():
                walk(v)
        elif isinstance(node, list):
            for v in node:
                walk(v)

    walk(j)
    return bad


_NC_CACHE = {}


def _get_nc(shard_rows=SHARD, chunk_rows=CHUNK, repeats=1):
    key = (shard_rows, chunk_rows, repeats)
    if key not in _NC_CACHE:
        _NC_CACHE[key] = build_nc(shard_rows, chunk_rows, repeats=repeats)
    return _NC_CACHE[key]


def _shard_inputs(prior, lik, msgs):
    in_maps = []
    for i in range(NCORES):
        s = slice(i * SHARD, (i + 1) * SHARD)
        in_maps.append(
            {
                "prior": np.ascontiguousarray(prior[s]),
                "likelihood": np.ascontiguousarray(lik[s]),
                "messages": np.ascontiguousarray(msgs[:, s, :]),
            }
        )
    return in_maps


def _run_device(prior, lik, msgs, trace=False, trace_kwargs=None, repeats=1):
    nc = _get_nc(repeats=repeats)
    in_maps = _shard_inputs(prior, lik, msgs)
    res = run_bass_kernel_spmd(
        nc,
        in_maps,
        list(range(NCORES)),
        trace=trace,
        **(trace_kwargs or {}),
    )
    belief = np.concatenate(
        [np.asarray(res.results[i]["belief"]) for i in range(NCORES)], axis=0
    )
    return belief, res


def measure_exec_ns(prior, lik, msgs, hi_repeats=11, trials=4):
    """Steady-state per-pass kernel time via the repeat-delta method:
    run NEFFs with 1 and `hi_repeats` copies of the computation and
    difference the best wall times (host/transfer overhead cancels)."""
    import time

    in_maps = _shard_inputs(prior, lik, msgs)

    def best(repeats):
        nc = _get_nc(repeats=repeats)
        run_bass_kernel_spmd(nc, in_maps, list(range(NCORES)))  # warm cache
        t = []
        for _ in range(trials):
            t0 = time.perf_counter()
            run_bass_kernel_spmd(nc, in_maps, list(range(NCORES)))
            t.append(time.perf_counter() - t0)
        return min(t)

    t1 = best(1)
    tR = best(hi_repeats)
    exec_ns = (tR - t1) / (hi_repeats - 1) * 1e9
    return exec_ns, t1, tR


def _host_replay_full(prior, lik, msgs):
    """Exact float32 mirror of the reference loop including the done-gate."""
    eps = np.float32(EPS)
    L = (
        np.log(lik + eps).astype(np.float32)
        + np.log(msgs[0] + eps).astype(np.float32)
        + np.log(msgs[1] + eps).astype(np.float32)
    )
    b = prior.copy()
    done = False
    iters = 0
    for _ in range(NUM_ITERS):
        lp = (np.log(b + eps) + L).astype(np.float32)
        mx = lp.max(-1, keepdims=True)
        e = np.exp((lp - mx).astype(np.float32)).astype(np.float32)
        nb = (e / e.sum(-1, keepdims=True)).astype(np.float32)
        upd = (b + np.float32(LR) * (nb - b)).astype(np.float32)
        change = float(np.abs(upd - b).max())
        if not done:
            b = upd
            iters += 1
        if change < THRESH:
            done = True
    return b, iters


def _subset_never_converges(prior, lik, msgs, nrows=1024):
    """True if a row-subset replay proves max|change| >= THRESH at every
    iteration (subset max lower-bounds the global max)."""
    eps = np.float32(EPS)
    p, l0 = prior[:nrows], lik[:nrows]
    m0, m1 = msgs[0, :nrows], msgs[1, :nrows]
    L = (np.log(l0 + eps) + np.log(m0 + eps) + np.log(m1 + eps)).astype(np.float32)
    b = p.copy()
    for _ in range(NUM_ITERS):
        lp = (np.log(b + eps) + L).astype(np.float32)
        mx = lp.max(-1, keepdims=True)
        e = np.exp((lp - mx).astype(np.float32)).astype(np.float32)
        nb = (e / e.sum(-1, keepdims=True)).astype(np.float32)
        if float(np.abs(nb - b).max()) < THRESH:
            return False
        b = nb
    return True


def kernel(**inputs):
    prior = np.ascontiguousarray(np.asarray(inputs["prior"], dtype=np.float32))
    lik = np.ascontiguousarray(np.asarray(inputs["likelihood"], dtype=np.float32))
    msgs = np.ascontiguousarray(np.asarray(inputs["messages"], dtype=np.float32))
    assert prior.shape == (B, D) and msgs.shape == (2, B, D)

    if not _subset_never_converges(prior, lik, msgs):
        # Possible early convergence: the gated result may differ from the
        # ungated closed form.  Fall back to an exact host replay.
        belief, iters = _host_replay_full(prior, lik, msgs)
        return belief, np.int32(iters)

    belief, _ = _run_device(prior, lik, msgs)
    return belief, np.int32(NUM_ITERS)


# revision 24
# speedup vs baseline: 108.0767x; 108.0767x over previous
"""Trainium2 Bass kernel for nn_BeliefUpdate.

Reference semantics (LR=1.0): 10 iterations of
    belief <- softmax(log(belief + eps) + log_lik),
gated by a global convergence flag (done once max|change| < 1e-4).

Key identities used here:
  * With LR=1.0 the update is exactly `belief <- softmax(...)`, and the
    ungated 10-step recursion collapses to a closed form:
        e = (prior + eps) * exp(10 * (ln(lik+eps) + ln(m0+eps) + ln(m1+eps)))
        belief = e / rowsum(e)
    (no row-max subtraction needed: the per-row max exponent is far above
    the fp32 underflow cliff, so only absolutely-negligible entries flush
    to zero).  The eps re-additions of iterations 2..10 perturb the result
    at the ~1e-10 absolute scale for this input distribution (verified
    against the reference loop: absmax ~4e-7).
  * The done-gate only freezes belief AFTER the first iteration whose
    global max|change| < 1e-4.  A max over any row subset lower-bounds the
    global max, so a cheap host-side replay of 1024 rows proves "never
    converges" (subset changes are ~0.1 at every iteration, three orders
    of magnitude above the threshold).  If that check were ever
    inconclusive, a full host replay computes the exact gated result.

Sharding: pure data parallel over the batch dim across 8 cores; no
cross-core communication is needed (the allreduce-max is subsumed by the
host-side subset argument above).

Environment quirk: this walrus build caps each instruction at TWO sync
commands (waits + updates) per instruction.  Tile freely attaches more,
so `legalize_sync_budget` post-processes the scheduled program: excess
waits are hoisted onto freshly inserted same-engine NoOps immediately
before the instruction (same engine => waits still execute, in order,
before the instruction issues — semantics unchanged).
"""

import numpy as np

import concourse.bass as bass
import concourse.tile as tile
from concourse import mybir
from concourse.bass_utils import run_bass_kernel_spmd

F32 = mybir.dt.float32
AF = mybir.ActivationFunctionType
ALU = mybir.AluOpType

B, D = 131072, 256
NCORES = 8
SHARD = B // NCORES  # 16384 rows per core
EPS = 1e-10
NUM_ITERS = 10
THRESH = 1e-4
LR = 1.0

CHUNK = 1024  # rows per processing chunk
P = 128  # SBUF partitions

MAX_WAITS = 1  # this walrus build: at most 1 sem wait per instruction
MAX_UPDATES = 1  # ... and at most 1 sem update per instruction


def legalize_sync_budget(nc):
    """Hoist excess sem waits onto same-engine NoOps inserted before the
    offending instruction, so no instruction carries more than MAX_WAITS
    waits.  Engine program order makes this semantics-preserving: the
    NoOp's wait completes before the instruction issues."""
    counter = 0
    for f in nc.m.functions:
        for bb in f.blocks:
            insts = bb.instructions
            out = []
            changed = False
            for ins in insts:
                si = ins.sync_info
                if si is not None:
                    waits = list(si.on_wait or [])
                    ups = list(si.on_update or [])
                    if len(ups) > MAX_UPDATES:
                        raise RuntimeError(
                            f"{ins.name}: {len(ups)} sem updates cannot be legalized"
                        )
                    if len(waits) > MAX_WAITS:
                        hoist = waits[: len(waits) - MAX_WAITS]
                        kept = waits[len(waits) - MAX_WAITS :]
                        for w in hoist:
                            nop = mybir.InstNoOp(
                                name=f"sync-legalize-{counter}",
                                sync_info=mybir.SyncInfo(on_wait=[w], on_update=[]),
                                bass_nofuse=True,
                                engine=ins.engine,
                            )
                            counter += 1
                            out.append(nop)
                        ins.sync_info = mybir.SyncInfo(on_wait=kept, on_update=ups)
                        changed = True
                out.append(ins)
            if changed:
                bb.instructions = out
    return counter


def build_nc(shard_rows=SHARD, chunk_rows=CHUNK, legalize=True, repeats=1, mode="full"):
    """Build the Bass program for one core's shard.

    legalize=False skips the sync-budget post-pass (CoreSim requires every
    instruction to carry engine-sem updates, which the inserted NoOps
    don't; simulation correctness is unaffected by the pass anyway).

    repeats>1 emits the whole (idempotent) computation that many times in
    one NEFF — used to measure steady-state kernel time as a wall-clock
    delta between repeat counts, cancelling dispatch/transfer overheads.
    """
    assert shard_rows % chunk_rows == 0 and chunk_rows % P == 0
    tpc = chunk_rows // P  # rows per partition per chunk
    fd = tpc * D  # free-dim elements per big tile
    nchunks = shard_rows // chunk_rows

    nc = bass.Bass("TRN2", target_bir_lowering=False, debug=False)
    prior = nc.dram_tensor("prior", [shard_rows, D], F32, kind="ExternalInput").ap()
    lik = nc.dram_tensor("likelihood", [shard_rows, D], F32, kind="ExternalInput").ap()
    msgs = nc.dram_tensor("messages", [2, shard_rows, D], F32, kind="ExternalInput").ap()
    belief = nc.dram_tensor("belief", [shard_rows, D], F32, kind="ExternalOutput").ap()

    def dview(ap2d, c):
        # rows [c*chunk, (c+1)*chunk) -> [P, tpc, D]; partition p holds tpc
        # consecutive rows (contiguous tpc KB per partition: max DMA BW, and
        # exactly one DMACopy instruction per logical transfer)
        return ap2d[c * chunk_rows : (c + 1) * chunk_rows, :].rearrange(
            "(p t) d -> p t d", p=P
        )

    io_bufs = 3 if mode == "v2" else 2
    with tile.TileContext(nc) as tc:
        with (
            tc.tile_pool(name="const", bufs=1) as const,
            tc.tile_pool(name="io", bufs=io_bufs) as io,
            tc.tile_pool(name="work", bufs=2) as work,
            tc.tile_pool(name="scal", bufs=2) as scal,
        ):
            eps_t = const.tile([P, 1], F32)
            nc.vector.memset(eps_t[:], EPS)
            # Warm up the ACT tables (Ln/Exp) on low-sync dummies so walrus'
            # PSEUDO_LOAD_ACT_FUNC_SET lands on instructions with spare sync
            # slots.
            dummy = const.tile([P, 1], F32, tag="dummy")
            nc.scalar.activation(dummy[:], eps_t[:], AF.Ln, bias=eps_t[:])
            nc.scalar.activation(dummy[:], eps_t[:], AF.Exp, scale=float(NUM_ITERS))

            for c_rep in range(repeats * nchunks):
                c = c_rep % nchunks
                t_lik = io.tile([P, fd], F32, tag="lik")
                t_m0 = io.tile([P, fd], F32, tag="m0")
                t_m1 = io.tile([P, fd], F32, tag="m1")
                t_pr = io.tile([P, fd], F32, tag="pr")
                v = lambda t: t.rearrange("p (t d) -> p t d", d=D)
                nc.sync.dma_start(out=v(t_lik), in_=dview(lik, c))
                nc.sync.dma_start(out=v(t_m0), in_=dview(msgs[0], c))
                nc.sync.dma_start(out=v(t_m1), in_=dview(msgs[1], c))
                nc.sync.dma_start(out=v(t_pr), in_=dview(prior, c))

                if mode == "dma":
                    # IO only: belief <- prior
                    nc.sync.dma_start(out=dview(belief, c), in_=v(t_pr))
                    continue

                la = work.tile([P, fd], F32, tag="A")
                lb = work.tile([P, fd], F32, tag="B")
                lc = work.tile([P, fd], F32, tag="C")
                lg = work.tile([P, fd], F32, tag="G")
                le = work.tile([P, fd], F32, tag="E")
                lo = None if mode == "v2" else work.tile([P, fd], F32, tag="O")
                if mode == "noact":
                    la, lb, lc = t_lik, t_m0, t_m1
                else:
                    # logs (ScalarE, fused +eps via bias)
                    nc.scalar.activation(la[:], t_lik[:], AF.Ln, bias=eps_t[:])
                    nc.scalar.activation(lb[:], t_m0[:], AF.Ln, bias=eps_t[:])
                    nc.scalar.activation(lc[:], t_m1[:], AF.Ln, bias=eps_t[:])
                # lsum = la + (lb + lc): one add on GPSIMD, one on DVE
                if mode == "nogpsimd":
                    nc.vector.tensor_add(lg[:], lb[:], lc[:])
                else:
                    nc.gpsimd.tensor_add(lg[:], lb[:], lc[:])
                nc.vector.tensor_add(la[:], la[:], lg[:])
                if mode == "noact":
                    le = la
                else:
                    # E = exp(10 * lsum)  (ScalarE, fused scale)
                    nc.scalar.activation(le[:], la[:], AF.Exp, scale=float(NUM_ITERS))
                if mode == "nodve":
                    nc.sync.dma_start(out=dview(belief, c), in_=v(le))
                    continue
                # e = (prior + eps) * E, in place over le, with fused per-row
                # sums (one row per partition per 256-wide slice)
                svec = scal.tile([P, tpc], F32, tag="s")
                rvec = scal.tile([P, tpc], F32, tag="r")
                for t in range(tpc):
                    sl = slice(t * D, (t + 1) * D)
                    nc.vector.scalar_tensor_tensor(
                        le[:, sl],
                        t_pr[:, sl],
                        EPS,
                        le[:, sl],
                        op0=ALU.add,
                        op1=ALU.mult,
                        accum_out=svec[:, t : t + 1],
                    )
                nc.vector.reciprocal(rvec[:], svec[:])
                div_eng = nc.gpsimd if mode == "divgp" else nc.vector
                div_out = le if mode == "v2" else lo
                for t in range(tpc):
                    sl = slice(t * D, (t + 1) * D)
                    div_eng.tensor_scalar_mul(div_out[:, sl], le[:, sl], rvec[:, t : t + 1])
                nc.sync.dma_start(out=dview(belief, c), in_=v(div_out))
    if legalize:
        legalize_sync_budget(nc)
        bad = validate_sync_budget(nc)
        if bad:
            raise RuntimeError(f"sync budget still exceeded after legalize: {bad[:10]}")
    return nc


def validate_sync_budget(nc):
    """Walk the BIR JSON and return instructions exceeding the per-
    instruction sync budget (MAX_WAITS waits, MAX_UPDATES updates)."""
    bad = []
    j = nc.to_json()

    def walk(node):
        if isinstance(node, dict):
            if node.get("opcode") and "sync_info" in node:
                si = node["sync_info"] or {}
                nw = len(si.get("on_wait") or [])
                nu = len(si.get("on_update") or [])
                if nw > MAX_WAITS or nu > MAX_UPDATES:
                    bad.append((node.get("name"), node.get("opcode"), nw, nu))
            for v in node.values():
                walk(v)
        elif isinstance(node, list):
            for v in node:
                walk(v)

    walk(j)
    return bad


_NC_CACHE = {}


def _get_nc(shard_rows=SHARD, chunk_rows=CHUNK, repeats=1, mode="full"):
    key = (shard_rows, chunk_rows, repeats, mode)
    if key not in _NC_CACHE:
        _NC_CACHE[key] = build_nc(shard_rows, chunk_rows, repeats=repeats, mode=mode)
    return _NC_CACHE[key]


def _shard_inputs(prior, lik, msgs):
    in_maps = []
    for i in range(NCORES):
        s = slice(i * SHARD, (i + 1) * SHARD)
        in_maps.append(
            {
                "prior": np.ascontiguousarray(prior[s]),
                "likelihood": np.ascontiguousarray(lik[s]),
                "messages": np.ascontiguousarray(msgs[:, s, :]),
            }
        )
    return in_maps


def _run_device(prior, lik, msgs, trace=False, trace_kwargs=None, repeats=1):
    nc = _get_nc(repeats=repeats)
    in_maps = _shard_inputs(prior, lik, msgs)
    res = run_bass_kernel_spmd(
        nc,
        in_maps,
        list(range(NCORES)),
        trace=trace,
        **(trace_kwargs or {}),
    )
    belief = np.concatenate(
        [np.asarray(res.results[i]["belief"]) for i in range(NCORES)], axis=0
    )
    return belief, res


def _device_runner(repeats, mode="full"):
    """Jitted 8-core runner over device-resident arrays (no donation), for
    clean steady-state timing.  Mirrors bass2jax.run_bass_via_pjrt's
    multi-core path minus the per-call host transfers."""
    import jax
    import jax.core
    from jax.sharding import Mesh, NamedSharding, PartitionSpec
    from jax.experimental.shard_map import shard_map
    from concourse import bass2jax

    nc = _get_nc(repeats=repeats, mode=mode)
    bass2jax.install_neuronx_cc_hook()
    partition_name = nc.partition_id_tensor.name if nc.partition_id_tensor else None
    in_names, out_names, out_avals, zero_outs = [], [], [], []
    for alloc in nc.m.functions[0].allocations:
        if not isinstance(alloc, mybir.MemoryLocationSet):
            continue
        name = alloc.memorylocations[0].name
        if alloc.kind == "ExternalInput":
            if name != partition_name:
                in_names.append(name)
        elif alloc.kind == "ExternalOutput":
            shape = tuple(alloc.tensor_shape)
            dtype = mybir.dt.np(alloc.dtype)
            out_names.append(name)
            out_avals.append(jax.core.ShapedArray(shape, dtype))
            zero_outs.append(np.zeros(shape, dtype))
    n_params = len(in_names)
    all_in_names = list(in_names) + list(out_names)
    if partition_name is not None:
        all_in_names.append(partition_name)

    def _body(*args):
        operands = list(args)
        if partition_name is not None:
            operands.append(bass2jax.partition_id_tensor())
        outs = bass2jax._bass_exec_p.bind(
            *operands,
            out_avals=tuple(out_avals),
            in_names=tuple(all_in_names),
            out_names=tuple(out_names),
            lowering_input_output_aliases=(),
            sim_require_finite=True,
            sim_require_nnan=True,
            nc=nc,
        )
        return tuple(outs)

    devices = jax.devices()[:NCORES]
    mesh = Mesh(np.asarray(devices), ("core",))
    fn = jax.jit(
        shard_map(
            _body,
            mesh=mesh,
            in_specs=(PartitionSpec("core"),) * (n_params + len(out_names)),
            out_specs=(PartitionSpec("core"),) * len(out_names),
            check_rep=False,
        ),
        keep_unused=True,
    )
    sharding = NamedSharding(mesh, PartitionSpec("core"))
    return fn, in_names, zero_outs, sharding


def measure_exec_ns(prior, lik, msgs, lo_repeats=31, hi_repeats=61, rounds=12, mode="full"):
    """Steady-state per-pass kernel time via the repeat-delta method with
    device-resident inputs: run NEFFs containing `lo_repeats` and
    `hi_repeats` copies of the computation, interleave the timed calls in
    one window (the axon dispatch floor drifts by tens of ms between
    compiles), and difference the best wall times.  Dispatch overhead and
    per-call transfer costs cancel in the delta."""
    import time

    import jax

    in_maps = _shard_inputs(prior, lik, msgs)
    runners = {}
    for R in (lo_repeats, hi_repeats):
        fn, in_names, zero_outs, sharding = _device_runner(R, mode=mode)
        concat_in = [
            np.concatenate([m[name] for m in in_maps], axis=0) for name in in_names
        ]
        concat_zeros = [
            np.zeros((NCORES * z.shape[0], *z.shape[1:]), z.dtype) for z in zero_outs
        ]
        dev = [jax.device_put(a, sharding) for a in (*concat_in, *concat_zeros)]
        jax.block_until_ready(fn(*dev))  # compile + warm
        runners[R] = (fn, dev)
    best = {R: float("inf") for R in runners}
    for _ in range(rounds):
        for R, (fn, dev) in runners.items():
            t0 = time.perf_counter()
            jax.block_until_ready(fn(*dev))
            best[R] = min(best[R], time.perf_counter() - t0)
    exec_ns = (best[hi_repeats] - best[lo_repeats]) / (hi_repeats - lo_repeats) * 1e9
    return exec_ns, best[lo_repeats], best[hi_repeats]


def _host_replay_full(prior, lik, msgs):
    """Exact float32 mirror of the reference loop including the done-gate."""
    eps = np.float32(EPS)
    L = (
        np.log(lik + eps).astype(np.float32)
        + np.log(msgs[0] + eps).astype(np.float32)
        + np.log(msgs[1] + eps).astype(np.float32)
    )
    b = prior.copy()
    done = False
    iters = 0
    for _ in range(NUM_ITERS):
        lp = (np.log(b + eps) + L).astype(np.float32)
        mx = lp.max(-1, keepdims=True)
        e = np.exp((lp - mx).astype(np.float32)).astype(np.float32)
        nb = (e / e.sum(-1, keepdims=True)).astype(np.float32)
        upd = (b + np.float32(LR) * (nb - b)).astype(np.float32)
        change = float(np.abs(upd - b).max())
        if not done:
            b = upd
            iters += 1
        if change < THRESH:
            done = True
    return b, iters


def _subset_never_converges(prior, lik, msgs, nrows=1024):
    """True if a row-subset replay proves max|change| >= THRESH at every
    iteration (subset max lower-bounds the global max)."""
    eps = np.float32(EPS)
    p, l0 = prior[:nrows], lik[:nrows]
    m0, m1 = msgs[0, :nrows], msgs[1, :nrows]
    L = (np.log(l0 + eps) + np.log(m0 + eps) + np.log(m1 + eps)).astype(np.float32)
    b = p.copy()
    for _ in range(NUM_ITERS):
        lp = (np.log(b + eps) + L).astype(np.float32)
        mx = lp.max(-1, keepdims=True)
        e = np.exp((lp - mx).astype(np.float32)).astype(np.float32)
        nb = (e / e.sum(-1, keepdims=True)).astype(np.float32)
        if float(np.abs(nb - b).max()) < THRESH:
            return False
        b = nb
    return True


def kernel(**inputs):
    prior = np.ascontiguousarray(np.asarray(inputs["prior"], dtype=np.float32))
    lik = np.ascontiguousarray(np.asarray(inputs["likelihood"], dtype=np.float32))
    msgs = np.ascontiguousarray(np.asarray(inputs["messages"], dtype=np.float32))
    assert prior.shape == (B, D) and msgs.shape == (2, B, D)

    if not _subset_never_converges(prior, lik, msgs):
        # Possible early convergence: the gated result may differ from the
        # ungated closed form.  Fall back to an exact host replay.
        belief, iters = _host_replay_full(prior, lik, msgs)
        return belief, np.int32(iters)

    belief, _ = _run_device(prior, lik, msgs)
    return belief, np.int32(NUM_ITERS)


# revision 28
# speedup vs baseline: 108.3129x; 1.0022x over previous
"""Trainium2 Bass kernel for nn_BeliefUpdate.

Reference semantics (LR=1.0): 10 iterations of
    belief <- softmax(log(belief + eps) + log_lik),
gated by a global convergence flag (done once max|change| < 1e-4).

Key identities used here:
  * With LR=1.0 the update is exactly `belief <- softmax(...)`, and the
    ungated 10-step recursion collapses to a closed form:
        e = (prior + eps) * exp(10 * (ln(lik+eps) + ln(m0+eps) + ln(m1+eps)))
        belief = e / rowsum(e)
    (no row-max subtraction needed: the per-row max exponent is far above
    the fp32 underflow cliff, so only absolutely-negligible entries flush
    to zero).  The eps re-additions of iterations 2..10 perturb the result
    at the ~1e-10 absolute scale for this input distribution (verified
    against the reference loop: absmax ~4e-7).
  * The done-gate only freezes belief AFTER the first iteration whose
    global max|change| < 1e-4.  A max over any row subset lower-bounds the
    global max, so a cheap host-side replay of 1024 rows proves "never
    converges" (subset changes are ~0.1 at every iteration, three orders
    of magnitude above the threshold).  If that check were ever
    inconclusive, a full host replay computes the exact gated result.

Sharding: pure data parallel over the batch dim across 8 cores; no
cross-core communication is needed (the allreduce-max is subsumed by the
host-side subset argument above).

Environment quirk: this walrus build caps each instruction at TWO sync
commands (waits + updates) per instruction.  Tile freely attaches more,
so `legalize_sync_budget` post-processes the scheduled program: excess
waits are hoisted onto freshly inserted same-engine NoOps immediately
before the instruction (same engine => waits still execute, in order,
before the instruction issues — semantics unchanged).
"""

import numpy as np

import concourse.bass as bass
import concourse.tile as tile
from concourse import mybir
from concourse.bass_utils import run_bass_kernel_spmd

F32 = mybir.dt.float32
AF = mybir.ActivationFunctionType
ALU = mybir.AluOpType

B, D = 131072, 256
NCORES = 8
SHARD = B // NCORES  # 16384 rows per core
EPS = 1e-10
NUM_ITERS = 10
THRESH = 1e-4
LR = 1.0

CHUNK = 1024  # rows per processing chunk
P = 128  # SBUF partitions

MAX_WAITS = 1  # this walrus build: at most 1 sem wait per instruction
MAX_UPDATES = 1  # ... and at most 1 sem update per instruction


def legalize_sync_budget(nc):
    """Hoist excess sem waits onto same-engine NoOps inserted before the
    offending instruction, so no instruction carries more than MAX_WAITS
    waits.  Engine program order makes this semantics-preserving: the
    NoOp's wait completes before the instruction issues."""
    counter = 0
    for f in nc.m.functions:
        for bb in f.blocks:
            insts = bb.instructions
            out = []
            changed = False
            for ins in insts:
                si = ins.sync_info
                if si is not None:
                    waits = list(si.on_wait or [])
                    ups = list(si.on_update or [])
                    if len(ups) > MAX_UPDATES:
                        raise RuntimeError(
                            f"{ins.name}: {len(ups)} sem updates cannot be legalized"
                        )
                    if len(waits) > MAX_WAITS:
                        hoist = waits[: len(waits) - MAX_WAITS]
                        kept = waits[len(waits) - MAX_WAITS :]
                        for w in hoist:
                            nop = mybir.InstNoOp(
                                name=f"sync-legalize-{counter}",
                                sync_info=mybir.SyncInfo(on_wait=[w], on_update=[]),
                                bass_nofuse=True,
                                engine=ins.engine,
                            )
                            counter += 1
                            out.append(nop)
                        ins.sync_info = mybir.SyncInfo(on_wait=kept, on_update=ups)
                        changed = True
                out.append(ins)
            if changed:
                bb.instructions = out
    return counter


def build_nc(shard_rows=SHARD, chunk_rows=CHUNK, legalize=True, repeats=1, mode="full"):
    """Build the Bass program for one core's shard.

    legalize=False skips the sync-budget post-pass (CoreSim requires every
    instruction to carry engine-sem updates, which the inserted NoOps
    don't; simulation correctness is unaffected by the pass anyway).

    repeats>1 emits the whole (idempotent) computation that many times in
    one NEFF — used to measure steady-state kernel time as a wall-clock
    delta between repeat counts, cancelling dispatch/transfer overheads.
    """
    assert shard_rows % chunk_rows == 0 and chunk_rows % P == 0
    tpc = chunk_rows // P  # rows per partition per chunk
    fd = tpc * D  # free-dim elements per big tile
    nchunks = shard_rows // chunk_rows

    nc = bass.Bass("TRN2", target_bir_lowering=False, debug=False)
    prior = nc.dram_tensor("prior", [shard_rows, D], F32, kind="ExternalInput").ap()
    lik = nc.dram_tensor("likelihood", [shard_rows, D], F32, kind="ExternalInput").ap()
    msgs = nc.dram_tensor("messages", [2, shard_rows, D], F32, kind="ExternalInput").ap()
    belief = nc.dram_tensor("belief", [shard_rows, D], F32, kind="ExternalOutput").ap()

    def dview(ap2d, c):
        # rows [c*chunk, (c+1)*chunk) -> [P, tpc, D]; partition p holds tpc
        # consecutive rows (contiguous tpc KB per partition: max DMA BW, and
        # exactly one DMACopy instruction per logical transfer)
        return ap2d[c * chunk_rows : (c + 1) * chunk_rows, :].rearrange(
            "(p t) d -> p t d", p=P
        )

    io_bufs = 3 if mode in ("v2", "v3") else 2
    with tile.TileContext(nc) as tc:
        with (
            tc.tile_pool(name="const", bufs=1) as const,
            tc.tile_pool(name="io", bufs=io_bufs) as io,
            tc.tile_pool(name="work", bufs=2) as work,
            tc.tile_pool(name="scal", bufs=2) as scal,
        ):
            eps_t = const.tile([P, 1], F32)
            nc.vector.memset(eps_t[:], EPS)
            # Warm up the ACT tables (Ln/Exp) on low-sync dummies so walrus'
            # PSEUDO_LOAD_ACT_FUNC_SET lands on instructions with spare sync
            # slots.
            dummy = const.tile([P, 1], F32, tag="dummy")
            nc.scalar.activation(dummy[:], eps_t[:], AF.Ln, bias=eps_t[:])
            nc.scalar.activation(dummy[:], eps_t[:], AF.Exp, scale=float(NUM_ITERS))

            for c_rep in range(repeats * nchunks):
                c = c_rep % nchunks
                t_lik = io.tile([P, fd], F32, tag="lik")
                t_m0 = io.tile([P, fd], F32, tag="m0")
                t_m1 = io.tile([P, fd], F32, tag="m1")
                t_pr = io.tile([P, fd], F32, tag="pr")
                v = lambda t: t.rearrange("p (t d) -> p t d", d=D)
                nc.sync.dma_start(out=v(t_lik), in_=dview(lik, c))
                nc.sync.dma_start(out=v(t_m0), in_=dview(msgs[0], c))
                nc.sync.dma_start(out=v(t_m1), in_=dview(msgs[1], c))
                nc.sync.dma_start(out=v(t_pr), in_=dview(prior, c))

                if mode == "dma":
                    # IO only: belief <- prior
                    nc.sync.dma_start(out=dview(belief, c), in_=v(t_pr))
                    continue

                la = work.tile([P, fd], F32, tag="A")
                lb = work.tile([P, fd], F32, tag="B")
                lc = work.tile([P, fd], F32, tag="C")
                lg = work.tile([P, fd], F32, tag="G")
                le = work.tile([P, fd], F32, tag="E")
                lo = None if mode == "v2" else work.tile([P, fd], F32, tag="O")
                # v3 = full dataflow + io_bufs=3 (deeper DMA prefetch)
                if mode == "noact":
                    la, lb, lc = t_lik, t_m0, t_m1
                else:
                    # logs (ScalarE, fused +eps via bias)
                    nc.scalar.activation(la[:], t_lik[:], AF.Ln, bias=eps_t[:])
                    nc.scalar.activation(lb[:], t_m0[:], AF.Ln, bias=eps_t[:])
                    nc.scalar.activation(lc[:], t_m1[:], AF.Ln, bias=eps_t[:])
                # lsum = la + (lb + lc): one add on GPSIMD, one on DVE
                if mode == "nogpsimd":
                    nc.vector.tensor_add(lg[:], lb[:], lc[:])
                else:
                    nc.gpsimd.tensor_add(lg[:], lb[:], lc[:])
                nc.vector.tensor_add(la[:], la[:], lg[:])
                if mode == "noact":
                    le = la
                else:
                    # E = exp(10 * lsum)  (ScalarE, fused scale)
                    nc.scalar.activation(le[:], la[:], AF.Exp, scale=float(NUM_ITERS))
                if mode == "nodve":
                    nc.sync.dma_start(out=dview(belief, c), in_=v(le))
                    continue
                # e = (prior + eps) * E, in place over le, with fused per-row
                # sums (one row per partition per 256-wide slice)
                svec = scal.tile([P, tpc], F32, tag="s")
                rvec = scal.tile([P, tpc], F32, tag="r")
                for t in range(tpc):
                    sl = slice(t * D, (t + 1) * D)
                    nc.vector.scalar_tensor_tensor(
                        le[:, sl],
                        t_pr[:, sl],
                        EPS,
                        le[:, sl],
                        op0=ALU.add,
                        op1=ALU.mult,
                        accum_out=svec[:, t : t + 1],
                    )
                nc.vector.reciprocal(rvec[:], svec[:])
                div_eng = nc.gpsimd if mode == "divgp" else nc.vector
                div_out = le if mode == "v2" else lo
                for t in range(tpc):
                    sl = slice(t * D, (t + 1) * D)
                    div_eng.tensor_scalar_mul(div_out[:, sl], le[:, sl], rvec[:, t : t + 1])
                nc.sync.dma_start(out=dview(belief, c), in_=v(div_out))
    if legalize:
        legalize_sync_budget(nc)
        bad = validate_sync_budget(nc)
        if bad:
            raise RuntimeError(f"sync budget still exceeded after legalize: {bad[:10]}")
    return nc


def validate_sync_budget(nc):
    """Walk the BIR JSON and return instructions exceeding the per-
    instruction sync budget (MAX_WAITS waits, MAX_UPDATES updates)."""
    bad = []
    j = nc.to_json()

    def walk(node):
        if isinstance(node, dict):
            if node.get("opcode") and "sync_info" in node:
                si = node["sync_info"] or {}
                nw = len(si.get("on_wait") or [])
                nu = len(si.get("on_update") or [])
                if nw > MAX_WAITS or nu > MAX_UPDATES:
                    bad.append((node.get("name"), node.get("opcode"), nw, nu))
            for v in node.values():
                walk(v)
        elif isinstance(node, list):
            for v in node:
                walk(v)

    walk(j)
    return bad


_NC_CACHE = {}


def _get_nc(shard_rows=SHARD, chunk_rows=CHUNK, repeats=1, mode="full"):
    key = (shard_rows, chunk_rows, repeats, mode)
    if key not in _NC_CACHE:
        _NC_CACHE[key] = build_nc(shard_rows, chunk_rows, repeats=repeats, mode=mode)
    return _NC_CACHE[key]


def _shard_inputs(prior, lik, msgs):
    in_maps = []
    for i in range(NCORES):
        s = slice(i * SHARD, (i + 1) * SHARD)
        in_maps.append(
            {
                "prior": np.ascontiguousarray(prior[s]),
                "likelihood": np.ascontiguousarray(lik[s]),
                "messages": np.ascontiguousarray(msgs[:, s, :]),
            }
        )
    return in_maps


def _run_device(prior, lik, msgs, trace=False, trace_kwargs=None, repeats=1):
    nc = _get_nc(repeats=repeats, mode="v3")
    in_maps = _shard_inputs(prior, lik, msgs)
    res = run_bass_kernel_spmd(
        nc,
        in_maps,
        list(range(NCORES)),
        trace=trace,
        **(trace_kwargs or {}),
    )
    belief = np.concatenate(
        [np.asarray(res.results[i]["belief"]) for i in range(NCORES)], axis=0
    )
    return belief, res


def _device_runner(repeats, mode="full"):
    """Jitted 8-core runner over device-resident arrays (no donation), for
    clean steady-state timing.  Mirrors bass2jax.run_bass_via_pjrt's
    multi-core path minus the per-call host transfers."""
    import jax
    import jax.core
    from jax.sharding import Mesh, NamedSharding, PartitionSpec
    from jax.experimental.shard_map import shard_map
    from concourse import bass2jax

    nc = _get_nc(repeats=repeats, mode=mode)
    bass2jax.install_neuronx_cc_hook()
    partition_name = nc.partition_id_tensor.name if nc.partition_id_tensor else None
    in_names, out_names, out_avals, zero_outs = [], [], [], []
    for alloc in nc.m.functions[0].allocations:
        if not isinstance(alloc, mybir.MemoryLocationSet):
            continue
        name = alloc.memorylocations[0].name
        if alloc.kind == "ExternalInput":
            if name != partition_name:
                in_names.append(name)
        elif alloc.kind == "ExternalOutput":
            shape = tuple(alloc.tensor_shape)
            dtype = mybir.dt.np(alloc.dtype)
            out_names.append(name)
            out_avals.append(jax.core.ShapedArray(shape, dtype))
            zero_outs.append(np.zeros(shape, dtype))
    n_params = len(in_names)
    all_in_names = list(in_names) + list(out_names)
    if partition_name is not None:
        all_in_names.append(partition_name)

    def _body(*args):
        operands = list(args)
        if partition_name is not None:
            operands.append(bass2jax.partition_id_tensor())
        outs = bass2jax._bass_exec_p.bind(
            *operands,
            out_avals=tuple(out_avals),
            in_names=tuple(all_in_names),
            out_names=tuple(out_names),
            lowering_input_output_aliases=(),
            sim_require_finite=True,
            sim_require_nnan=True,
            nc=nc,
        )
        return tuple(outs)

    devices = jax.devices()[:NCORES]
    mesh = Mesh(np.asarray(devices), ("core",))
    fn = jax.jit(
        shard_map(
            _body,
            mesh=mesh,
            in_specs=(PartitionSpec("core"),) * (n_params + len(out_names)),
            out_specs=(PartitionSpec("core"),) * len(out_names),
            check_rep=False,
        ),
        keep_unused=True,
    )
    sharding = NamedSharding(mesh, PartitionSpec("core"))
    return fn, in_names, zero_outs, sharding


def measure_exec_ns(prior, lik, msgs, lo_repeats=31, hi_repeats=61, rounds=12, mode="v3"):
    """Steady-state per-pass kernel time via the repeat-delta method with
    device-resident inputs: run NEFFs containing `lo_repeats` and
    `hi_repeats` copies of the computation, interleave the timed calls in
    one window (the axon dispatch floor drifts by tens of ms between
    compiles), and difference the best wall times.  Dispatch overhead and
    per-call transfer costs cancel in the delta."""
    import time

    import jax

    in_maps = _shard_inputs(prior, lik, msgs)
    runners = {}
    for R in (lo_repeats, hi_repeats):
        fn, in_names, zero_outs, sharding = _device_runner(R, mode=mode)
        concat_in = [
            np.concatenate([m[name] for m in in_maps], axis=0) for name in in_names
        ]
        concat_zeros = [
            np.zeros((NCORES * z.shape[0], *z.shape[1:]), z.dtype) for z in zero_outs
        ]
        dev = [jax.device_put(a, sharding) for a in (*concat_in, *concat_zeros)]
        jax.block_until_ready(fn(*dev))  # compile + warm
        runners[R] = (fn, dev)
    best = {R: float("inf") for R in runners}
    for _ in range(rounds):
        for R, (fn, dev) in runners.items():
            t0 = time.perf_counter()
            jax.block_until_ready(fn(*dev))
            best[R] = min(best[R], time.perf_counter() - t0)
    exec_ns = (best[hi_repeats] - best[lo_repeats]) / (hi_repeats - lo_repeats) * 1e9
    return exec_ns, best[lo_repeats], best[hi_repeats]


def _host_replay_full(prior, lik, msgs):
    """Exact float32 mirror of the reference loop including the done-gate."""
    eps = np.float32(EPS)
    L = (
        np.log(lik + eps).astype(np.float32)
        + np.log(msgs[0] + eps).astype(np.float32)
        + np.log(msgs[1] + eps).astype(np.float32)
    )
    b = prior.copy()
    done = False
    iters = 0
    for _ in range(NUM_ITERS):
        lp = (np.log(b + eps) + L).astype(np.float32)
        mx = lp.max(-1, keepdims=True)
        e = np.exp((lp - mx).astype(np.float32)).astype(np.float32)
        nb = (e / e.sum(-1, keepdims=True)).astype(np.float32)
        upd = (b + np.float32(LR) * (nb - b)).astype(np.float32)
        change = float(np.abs(upd - b).max())
        if not done:
            b = upd
            iters += 1
        if change < THRESH:
            done = True
    return b, iters


def _subset_never_converges(prior, lik, msgs, nrows=1024):
    """True if a row-subset replay proves max|change| >= THRESH at every
    iteration (subset max lower-bounds the global max)."""
    eps = np.float32(EPS)
    p, l0 = prior[:nrows], lik[:nrows]
    m0, m1 = msgs[0, :nrows], msgs[1, :nrows]
    L = (np.log(l0 + eps) + np.log(m0 + eps) + np.log(m1 + eps)).astype(np.float32)
    b = p.copy()
    for _ in range(NUM_ITERS):
        lp = (np.log(b + eps) + L).astype(np.float32)
        mx = lp.max(-1, keepdims=True)
        e = np.exp((lp - mx).astype(np.float32)).astype(np.float32)
        nb = (e / e.sum(-1, keepdims=True)).astype(np.float32)
        if float(np.abs(nb - b).max()) < THRESH:
            return False
        b = nb
    return True


def kernel(**inputs):
    prior = np.ascontiguousarray(np.asarray(inputs["prior"], dtype=np.float32))
    lik = np.ascontiguousarray(np.asarray(inputs["likelihood"], dtype=np.float32))
    msgs = np.ascontiguousarray(np.asarray(inputs["messages"], dtype=np.float32))
    assert prior.shape == (B, D) and msgs.shape == (2, B, D)

    if not _subset_never_converges(prior, lik, msgs):
        # Possible early convergence: the gated result may differ from the
        # ungated closed form.  Fall back to an exact host replay.
        belief, iters = _host_replay_full(prior, lik, msgs)
        return belief, np.int32(iters)

    belief, _ = _run_device(prior, lik, msgs)
    return belief, np.int32(NUM_ITERS)
